# revision 6
# baseline (speedup 1.0000x reference)
"""Distributed real SHT (spherical harmonic transform) for Trainium2, v2.

Computes, for x [1, 256, 361, 720] f32 and weights [361, 360, 361] f32:
    xf = 2*pi * rfft(x, axis=-1, norm='forward')[..., :361]
    out_re = einsum('bckm,mlk->bclm', Re(xf), weights)
    out_im = einsum('bckm,mlk->bclm', Im(xf), weights)
    return complex64 [1, 256, 360, 361]

Sharding: channels (dim 1) across 8 NeuronCores, 32 channels each.

Symmetries (validated against the reference arrays in the v1 kernel):
  * longitude fold: x[n]+/-x[720-n] halves the DFT contraction (361 rows).
  * latitude parity: P_l^m(-x) = (-1)^(l+m) P_l^m(x), so latitude pairs
    fold into even/odd branches of 181 folded latitudes; each branch
    feeds a disjoint set of output l's ((l+m)%2 == s).

v2 design — fused, no DRAM xf roundtrip:
  The kernel runs two fully independent passes, one per parity s. In a
  pass, stage A computes xf for that parity directly into SBUF in the
  layout stage B consumes: the DFT matmul uses the folded input xt as
  the STATIONARY operand ([128 lon, 96 lat] per (channel, lat-half)) and
  the DFT matrix F as the MOVING operand ([128 lon, 360 modes]), so PSUM
  holds [96 lat, 360 m] per (c, kh, ri) and the vector evict assembles
  xf[kh][96 lat, (ri, c, m)] with contiguous writes. Stage B then runs
  per-quad Legendre matmuls with xf slices ([96, 32c] strided cols) as
  stationary and the parity-s weight half as moving, accumulating the
  two lat-halves in PSUM, evicting bf16 outputs per octet of 8 quads.
  All DMA transfers carry multi-KB contiguous runs.
"""

import numpy as np
import ml_dtypes

NLAT = 361          # latitudes (k)
NLON = 720          # longitudes (n)
LMAX = 360          # output degree count (l = 0..359)
MMAX = 361          # rfft modes kept; m=360 output is all-zero (l<m)
C = 256
N_CORES = 8
C_LOC = C // N_CORES        # 32 channels per core
NPAD = 384                  # folded longitude (361 rows) padded to 3*128
NCH = 3                     # folded n split into 3 chunks of 128
KH = 180                    # latitude fold midpoint (k=180 self-paired)
KSP = 192                   # folded latitudes (181) padded to 2*96
MF = 360                    # modes computed on device (m=360 stays zero)
NQ = LMAX // 4              # 90 m-quads (m = 4q + m', m' in 0..3)

BF16 = ml_dtypes.bfloat16

_QL = [LMAX - 4 * q for q in range(NQ)]
_QLH = [L // 2 for L in _QL]
OCT = [(8 * o, min(8 * o + 8, NQ)) for o in range((NQ + 7) // 8)]
N_OCT = len(OCT)
# per-octet column geometry
_G = [sum(4 * _QLH[q] for q in range(a, b)) for a, b in OCT]   # wt cols/kh
_OTC = [sum(2 * _QLH[q] for q in range(a, b)) for a, b in OCT]  # ot cols
# intra-octet column offsets
_QG = []   # (q) -> col offset of quad q within wt tile (units of cols)
_QO = []   # (q) -> col offset of quad q within ot tile
for o, (a, b) in enumerate(OCT):
    g = 0
    t = 0
    for q in range(a, b):
        _QG.append(g)
        _QO.append(t)
        g += 4 * _QLH[q]
        t += 2 * _QLH[q]

# wq blob offsets keyed (s, oct, kh); ob offsets keyed (s, oct)
_WQ_OFF = {}
_off = 0
for _s in range(2):
    for _o in range(N_OCT):
        for _kh in range(2):
            _WQ_OFF[(_s, _o, _kh)] = _off
            _off += 96 * _G[_o]
WQ_TOTAL = _off
_OB_OFF = {}
_off = 0
for _s in range(2):
    for _o in range(N_OCT):
        _OB_OFF[(_s, _o)] = _off
        _off += 128 * _OTC[_o]
OB_TOTAL = _off

# xt: rows (s 2, ri 2, chalf 2, nch 3, p 128) = 3072, cols (c' 16, kh 2,
# kf 96) = 3072. par == ri (cos branch pairs with the symmetric fold).
XT_ROWS = 2 * 2 * 2 * NCH * 128
XT_COLS = 16 * 2 * 96

_CACHE = {}


def _build_bass(reps=1):
    import concourse.mybir as mybir
    import concourse.tile as tile
    from concourse import bacc
    from contextlib import nullcontext

    bf16 = mybir.dt.bfloat16
    f32 = mybir.dt.float32

    nc = bacc.Bacc("TRN2", target_bir_lowering=False, debug=False,
                   num_devices=N_CORES)

    xt_d = nc.dram_tensor("xt", [XT_ROWS, XT_COLS], bf16,
                          kind="ExternalInput")
    f_d = nc.dram_tensor("fm", [NCH * 128, 2 * MF], bf16,
                         kind="ExternalInput")
    wq_d = nc.dram_tensor("wq", [WQ_TOTAL], bf16, kind="ExternalInput")
    ob_d = nc.dram_tensor("ob", [OB_TOTAL], bf16, kind="ExternalOutput")

    with tile.TileContext(nc) as tc:
        with (
            tc.tile_pool(name="fpool", bufs=1) as fpool,
            tc.tile_pool(name="xtp", bufs=2) as xtp,
            tc.tile_pool(name="xfp", bufs=1) as xfp,
            tc.tile_pool(name="wtp", bufs=4) as wtp,
            tc.tile_pool(name="otp", bufs=2) as otp,
            tc.tile_pool(name="psA", bufs=4, space="PSUM") as psA,
            tc.tile_pool(name="psB", bufs=4, space="PSUM") as psB,
            tc.For_i(0, reps, 1) if reps > 1 else nullcontext(),
        ):
            # F resident: [128, nch 3, (ri 2, m 360)]
            f_tile = fpool.tile([128, NCH, 2 * MF], bf16)
            nc.sync.dma_start(
                f_tile[:],
                f_d[:].rearrange("(a p) f -> p a f", p=128),
            )

            for s in range(2):
                # xf for this parity, in stage-B layout:
                # [96 kf, ri 2, c 32, m 360] bf16 per lat-half kh
                # [96 kf, rc 64 = (ri 2, c 32), m 360]
                xf = [xfp.tile([96, 2 * C_LOC, MF], bf16, tag=f"xf{kh}",
                               name=f"xf{kh}")
                      for kh in range(2)]

                # ---------- stage A: DFT into SBUF xf ----------
                for ri in range(2):
                    for chf in range(2):
                        rb = (((s * 2 + ri) * 2 + chf) * NCH) * 128
                        xtq = xtp.tile([128, NCH, XT_COLS], bf16, tag="xt")
                        nc.sync.dma_start(
                            xtq[:],
                            xt_d[rb:rb + NCH * 128, :].rearrange(
                                "(a p) f -> p a f", p=128),
                        )
                        for cc in range(16):
                            c = chf * 16 + cc
                            for kh in range(2):
                                ps = psA.tile([96, MF], f32, tag="psA")
                                col0 = cc * 192 + kh * 96
                                for ncc in range(NCH):
                                    nc.tensor.matmul(
                                        ps[:],
                                        xtq[:, ncc, col0:col0 + 96],
                                        f_tile[:, ncc,
                                               ri * MF:(ri + 1) * MF],
                                        start=(ncc == 0),
                                        stop=(ncc == NCH - 1),
                                    )
                                if kh == 0:
                                    nc.vector.tensor_copy(
                                        xf[kh][:, ri * C_LOC + c, :], ps[:])
                                else:
                                    nc.scalar.copy(
                                        xf[kh][:, ri * C_LOC + c, :], ps[:])

                # ---------- stage B: Legendre contraction ----------
                for o in range(N_OCT):
                    q0, q1 = OCT[o]
                    wt = []
                    for kh in range(2):
                        w = wtp.tile([96, _G[o]], bf16, tag="wt")
                        woff = _WQ_OFF[(s, o, kh)]
                        nc.sync.dma_start(
                            w[:],
                            wq_d[woff:woff + 96 * _G[o]].rearrange(
                                "(p f) -> p f", p=96),
                        )
                        wt.append(w)
                    ot = otp.tile([128, _OTC[o]], bf16, tag="ot")
                    for q in range(q0, q1):
                        Lh = _QLH[q]
                        for mph in range(2):
                            ps = psB.tile([128, Lh], f32, tag="psB")
                            for mp2 in range(2):
                                mp = 2 * mph + mp2
                                m = 4 * q + mp
                                g0 = _QG[q] + mp * Lh
                                for kh in range(2):
                                    nc.tensor.matmul(
                                        ps[mp2 * 64:(mp2 + 1) * 64, :],
                                        xf[kh][:, :, m],
                                        wt[kh][:, g0:g0 + Lh],
                                        start=(kh == 0),
                                        stop=(kh == 1),
                                        tile_position=(0, mp2 * 64),
                                    )
                            nc.vector.tensor_copy(
                                ot[:, _QO[q] + mph * Lh:
                                   _QO[q] + (mph + 1) * Lh],
                                ps[:])
                    ooff = _OB_OFF[(s, o)]
                    nc.scalar.dma_start(
                        ob_d[ooff:ooff + 128 * _OTC[o]].rearrange(
                            "(p f) -> p f", p=128),
                        ot[:],
                    )

    nc.compile()
    return nc


def _dft_matrix():
    # folded longitude: n = 0..360; cos for re branch, -sin for im branch
    n = np.arange(NLON // 2 + 1, dtype=np.float64)[:, None]
    m = np.arange(MF, dtype=np.float64)[None, :]
    ang = 2.0 * np.pi * n * m / NLON
    coef = 2.0 * np.pi / NLON
    F = np.zeros((NPAD, 2, MF), dtype=np.float32)
    F[:NLON // 2 + 1, 0, :] = (coef * np.cos(ang)).astype(np.float32)
    F[:NLON // 2 + 1, 1, :] = (-coef * np.sin(ang)).astype(np.float32)
    return F.reshape(NPAD, 2 * MF).astype(BF16)


def _parity_ls(q, mp):
    """l-lists for quad q, mode m=4q+mp: index p means (l+m) % 2 == p."""
    m = 4 * q + mp
    l0 = 4 * q
    out = []
    for p in range(2):
        start = l0 if (l0 + m) % 2 == p else l0 + 1
        out.append(np.arange(start, LMAX, 2))
    return out


def _pack_weights(weights):
    # weights [MMAX, LMAX, NLAT] f32 -> per-(s, octet, kh) blob bf16:
    # block [96 kf, (q in octet: mp 4, Lh)]
    blob = np.empty(WQ_TOTAL, dtype=BF16)
    rev = np.arange(NLAT - 1, -1, -1)
    for s in range(2):
        sign = 1.0 - 2.0 * s
        for o, (a, b) in enumerate(OCT):
            arr = np.zeros((2, 96, _G[o]), dtype=np.float32)
            for q in range(a, b):
                Lh = _QLH[q]
                for mp in range(4):
                    m = 4 * q + mp
                    ls = _parity_ls(q, mp)[s]
                    Wm = weights[m][ls]                      # [Lh, 361]
                    Wf = np.zeros((Lh, KSP), dtype=np.float32)
                    Wf[:, :KH] = 0.5 * (Wm[:, :KH] + sign * Wm[:, rev[:KH]])
                    Wf[:, KH] = Wm[:, KH]
                    g0 = _QG[q] + mp * Lh
                    arr[0, :, g0:g0 + Lh] = Wf[:, :96].T
                    arr[1, :, g0:g0 + Lh] = Wf[:, 96:].T
            for kh in range(2):
                woff = _WQ_OFF[(s, o, kh)]
                blob[woff:woff + 96 * _G[o]] = \
                    arr[kh].astype(BF16).ravel()
    return blob


class _Runner:
    """jit(shard_map(bass_exec)) over the 8 cores; inputs stay resident as
    sharded jax arrays so repeated timed executions skip host transfer."""

    def __init__(self, nc):
        import jax
        import concourse.mybir as mybir
        from jax.experimental.shard_map import shard_map
        from jax.sharding import Mesh, PartitionSpec, NamedSharding
        from concourse.bass2jax import (
            _bass_exec_p, install_neuronx_cc_hook, partition_id_tensor)

        install_neuronx_cc_hook()
        self.jax = jax
        self.nc = nc
        part_name = (nc.partition_id_tensor.name
                     if nc.partition_id_tensor else None)
        in_names, out_names, out_avals, zero_outs = [], [], [], []
        for alloc in nc.m.functions[0].allocations:
            if not isinstance(alloc, mybir.MemoryLocationSet):
                continue
            name = alloc.memorylocations[0].name
            if alloc.kind == "ExternalInput":
                if name != part_name:
                    in_names.append(name)
            elif alloc.kind == "ExternalOutput":
                shape = tuple(alloc.tensor_shape)
                dtype = mybir.dt.np(alloc.dtype)
                out_names.append(name)
                out_avals.append(jax.core.ShapedArray(shape, dtype))
                zero_outs.append(np.zeros(shape, dtype))
        self.in_names = list(in_names)
        self.out_names = out_names
        self.out_avals = out_avals
        self.zero_outs = zero_outs
        all_names = in_names + out_names
        if part_name is not None:
            all_names = all_names + [part_name]

        def _body(*args):
            operands = list(args)
            if part_name is not None:
                operands.append(partition_id_tensor())
            outs = _bass_exec_p.bind(
                *operands,
                out_avals=tuple(out_avals),
                in_names=tuple(all_names),
                out_names=tuple(out_names),
                lowering_input_output_aliases=(),
                sim_require_finite=True,
                sim_require_nnan=True,
                nc=nc,
            )
            return tuple(outs)

        devices = jax.devices()[:N_CORES]
        mesh = Mesh(np.asarray(devices), ("core",))
        spec = PartitionSpec("core")
        n_args = len(in_names) + len(out_names)
        self.sharding = NamedSharding(mesh, spec)
        self.fn = jax.jit(
            shard_map(_body, mesh=mesh,
                      in_specs=(spec,) * n_args,
                      out_specs=(spec,) * len(out_names),
                      check_rep=False),
            keep_unused=True,
        )

    def device_args(self, in_maps):
        jax = self.jax
        args = []
        for name in self.in_names:
            cat = np.concatenate([m[name] for m in in_maps], axis=0)
            args.append(jax.device_put(cat, self.sharding))
        for z in self.zero_outs:
            cat = np.zeros((N_CORES * z.shape[0], *z.shape[1:]), z.dtype)
            args.append(jax.device_put(cat, self.sharding))
        return args

    def execute(self, args):
        outs = self.fn(*args)
        self.jax.block_until_ready(outs)
        return outs

    def run(self, in_maps):
        outs = self.execute(self.device_args(in_maps))
        results = []
        for c in range(N_CORES):
            r = {}
            for i, name in enumerate(self.out_names):
                full = np.asarray(outs[i])
                r[name] = full.reshape(N_CORES, *self.out_avals[i].shape)[c]
            results.append(r)
        return results


def get_runner(reps=1, stage="AB"):
    key = ("runner", reps)
    if key not in _CACHE:
        _CACHE[key] = _Runner(_build_bass(reps))
    return _CACHE[key]


def prepare_in_maps(x, weights):
    if "F" not in _CACHE:
        _CACHE["F"] = _dft_matrix()
    F = _CACHE["F"]
    x = np.asarray(x, dtype=np.float32)
    weights = np.asarray(weights, dtype=np.float32)
    wq = _pack_weights(weights)

    nh = NLON // 2  # 360
    in_maps = []
    for p in range(N_CORES):
        xs = x[0, p * C_LOC:(p + 1) * C_LOC]          # [32, 361, 720]
        xn = xs.transpose(2, 1, 0)                    # [720 n, 361 k, 32 c]
        # latitude fold: [720, s 2, 192, 32]
        xkf = np.zeros((NLON, 2, KSP, C_LOC), dtype=np.float32)
        for s in range(2):
            sign = 1.0 - 2.0 * s
            xkf[:, s, :KH] = xn[:, :KH] + sign * xn[:, NLAT - 1:KH:-1]
            xkf[:, s, KH] = xn[:, KH]
        # longitude fold: [par 2, npad 384, s 2, 192, 32]
        xt = np.zeros((2, NPAD, 2, KSP, C_LOC), dtype=np.float32)
        xt[0, 0] = xkf[0]
        xt[0, 1:nh] = xkf[1:nh] + xkf[:nh:-1]
        xt[0, nh] = xkf[nh]
        xt[1, 1:nh] = xkf[1:nh] - xkf[:nh:-1]
        # repack to [s, ri(par), chalf, nch, p, c' 16, kh, kf 96]
        xr = xt.reshape(2, NCH, 128, 2, 2, 96, C_LOC)
        # dims: (par, nch, p, s, kh, kf, c)
        xr = xr.transpose(3, 0, 1, 2, 6, 4, 5)
        # now (s, par, nch, p, c 32, kh, kf)
        xr = xr.reshape(2, 2, NCH, 128, 2, 16, 2, 96)
        # (s, par, nch, p, chalf, c' 16, kh, kf) -> move chalf before nch
        xr = xr.transpose(0, 1, 4, 2, 3, 5, 6, 7)
        xpk = np.ascontiguousarray(xr).reshape(XT_ROWS, XT_COLS)
        in_maps.append({
            "xt": xpk.astype(BF16),
            "fm": F,
            "wq": wq,
        })
    return in_maps


def unpack_results(results):
    out_re = np.zeros((C, LMAX, MMAX), dtype=np.float32)
    out_im = np.zeros((C, LMAX, MMAX), dtype=np.float32)
    for p in range(N_CORES):
        ob = results[p]["ob"]
        c0 = p * C_LOC
        for s in range(2):
            for o, (a, b) in enumerate(OCT):
                blk = ob[_OB_OFF[(s, o)]:
                         _OB_OFF[(s, o)] + 128 * _OTC[o]]
                blk = blk.reshape(2, 2, C_LOC, _OTC[o]).astype(np.float32)
                for q in range(a, b):
                    Lh = _QLH[q]
                    t0 = _QO[q]
                    for mph in range(2):
                        for mp2 in range(2):
                            m = 4 * q + 2 * mph + mp2
                            ls = _parity_ls(q, 2 * mph + mp2)[s]
                            cols = slice(t0 + mph * Lh, t0 + (mph + 1) * Lh)
                            out_re[c0:c0 + C_LOC, ls, m] = \
                                blk[mp2, 0, :, cols]
                            out_im[c0:c0 + C_LOC, ls, m] = \
                                blk[mp2, 1, :, cols]
    out = (out_re + 1j * out_im).astype(np.complex64)
    return out.reshape(1, C, LMAX, MMAX)


def kernel(x, weights):
    runner = get_runner()
    in_maps = prepare_in_maps(x, weights)
    results = runner.run(in_maps)
    return unpack_results(results)
